# revision 1
# baseline (speedup 1.0000x reference)
"""RNN-T joint network (dense MLP) Trainium2 Bass kernel.

Math (per batch row n):
    h = relu(f @ W1t.T + g @ W1p.T + b1t + b1p)    # [N, 512]
    y = h @ W2.T + b2                              # [N, 29]

Strategy: data-parallel over batch N=32768 across 8 NeuronCores (4096
rows/core); weights replicated.  Host-side layout prep: x = concat(f, g)
transposed to [K, N] so contraction K sits on SBUF partitions with zero
on-device transposes; K padded 1344 -> 1408 (11 full 128-tiles).
On-device: h.T[j, n] in PSUM via 11 accumulating matmuls per j-tile
(float32r, 1 cyc/row), bias+relu via ScalarE, second matmul contracts
j into y.T[29, n], bias via ScalarE, DMA out.  Host transposes y back.
"""

import numpy as np

import concourse.bacc as bacc
import concourse.bass as bass  # noqa: F401
import concourse.mybir as mybir
from concourse import tile
from concourse.bass_utils import run_bass_kernel_spmd

TRANS_H, PRED_H, JOINT_H, NUM_LABELS = 1024, 320, 512, 29
BATCH = 32768
N_CORES = 8
N_PER_CORE = BATCH // N_CORES          # 4096
K_TOTAL = TRANS_H + PRED_H             # 1344
K_PAD = 1408                           # 11 * 128
K_TILES = K_PAD // 128                 # 11
J_TILES = JOINT_H // 128               # 4
N_CHUNK = 512                          # fp32 moving-operand / PSUM-bank limit
N_CHUNKS = N_PER_CORE // N_CHUNK       # 8

F32 = mybir.dt.float32
F32R = mybir.dt.float32r

_NC_CACHE = {}


def _build_bass():
    """Build the single-core Bass program (same NEFF runs SPMD on 8 cores)."""
    nc = bacc.Bacc(None)

    xT = nc.dram_tensor("xT", [K_PAD, N_PER_CORE], F32R, kind="ExternalInput")
    w1 = nc.dram_tensor("w1", [K_PAD, JOINT_H], F32R, kind="ExternalInput")
    b1 = nc.dram_tensor("b1", [JOINT_H, 1], F32, kind="ExternalInput")
    w2T = nc.dram_tensor("w2T", [JOINT_H, NUM_LABELS], F32R, kind="ExternalInput")
    b2 = nc.dram_tensor("b2", [NUM_LABELS, 1], F32, kind="ExternalInput")
    yT = nc.dram_tensor("yT", [NUM_LABELS, N_PER_CORE], F32, kind="ExternalOutput")

    # views with the k-tile index explicit: row (k*128 + p) -> [p, k, ...]
    xT3 = xT.rearrange("(k p) n -> p k n", p=128)     # [128, K_TILES, N]
    w13 = w1.rearrange("(k p) j -> p k j", p=128)     # [128, K_TILES, JOINT_H]

    # k-tile split for each x-chunk DMA (2 pieces -> pipeline fill + issue amortized)
    K_SPLITS = [(0, 6), (6, K_TILES)]
    # finer pieces for the pipeline-fill chunk so the first matmuls start early
    K_SPLITS_FILL = [(0, 2), (2, 4), (4, 6), (6, 8), (8, K_TILES)]

    with tile.TileContext(nc) as tc:
        with (
            tc.tile_pool(name="consts", bufs=1) as consts,
            tc.tile_pool(name="xpool", bufs=3) as xpool,
            tc.tile_pool(name="hpool", bufs=2) as hpool,
            tc.tile_pool(name="opool", bufs=2) as opool,
            tc.tile_pool(name="psum_h", bufs=6, space="PSUM") as psum_h,
            tc.tile_pool(name="psum_y", bufs=2, space="PSUM") as psum_y,
        ):
            # ---- replicated constants (ACT-ring DMAs; x rides the SP ring) ----
            w1_sb = consts.tile([128, K_TILES, JOINT_H], F32R, name="w1_sb", tag="w1")
            for (ka, kb) in K_SPLITS_FILL:
                nc.scalar.dma_start(out=w1_sb[:, ka:kb, :], in_=w13[:, ka:kb, :])
            w2_sb = consts.tile([128, J_TILES, NUM_LABELS], F32R, name="w2_sb", tag="w2")
            nc.scalar.dma_start(
                out=w2_sb,
                in_=w2T.rearrange("(j p) l -> p j l", p=128),
            )
            b1_sb = consts.tile([128, J_TILES], F32, name="b1_sb", tag="b1")
            nc.scalar.dma_start(
                out=b1_sb, in_=b1.rearrange("(j p) o -> p (j o)", p=128)
            )
            b2_sb = consts.tile([NUM_LABELS, 1], F32, name="b2_sb", tag="b2")
            nc.scalar.dma_start(out=b2_sb, in_=b2[:, :])

            # ---- main loop over batch chunks of 512 ----
            for c in range(N_CHUNKS):
                n0 = c * N_CHUNK
                x_sb = xpool.tile([128, K_TILES, N_CHUNK], F32R, name="x_sb", tag="x")
                for (ka, kb) in (K_SPLITS_FILL if c <= 2 else K_SPLITS):
                    nc.sync.dma_start(
                        out=x_sb[:, ka:kb, :], in_=xT3[:, ka:kb, n0:n0 + N_CHUNK]
                    )

                h_tiles = []
                for j in range(J_TILES):
                    ph = psum_h.tile([128, N_CHUNK], F32, name=f"ph_{j}", tag="ph")
                    for k in range(K_TILES):
                        nc.tensor.matmul(
                            ph,
                            lhsT=w1_sb[:, k, j * 128:(j + 1) * 128],
                            rhs=x_sb[:, k, :],
                            start=(k == 0),
                            stop=(k == K_TILES - 1),
                        )
                    h_sb = hpool.tile([128, N_CHUNK], F32R, name=f"h_{j}", tag=f"h_{j}")
                    nc.scalar.activation(
                        h_sb, ph, mybir.ActivationFunctionType.Relu,
                        bias=b1_sb[:, j:j + 1],
                    )
                    h_tiles.append(h_sb)

                py = psum_y.tile([NUM_LABELS, N_CHUNK], F32, name="py", tag="py")
                for j in range(J_TILES):
                    nc.tensor.matmul(
                        py,
                        lhsT=w2_sb[:, j, :],
                        rhs=h_tiles[j],
                        start=(j == 0),
                        stop=(j == J_TILES - 1),
                    )
                y_sb = opool.tile([NUM_LABELS, N_CHUNK], F32, name="y_sb", tag="y")
                nc.scalar.activation(
                    y_sb, py, mybir.ActivationFunctionType.Identity, bias=b2_sb
                )
                nc.scalar.dma_start(out=yT[:, n0:n0 + N_CHUNK], in_=y_sb)

    nc.finalize()
    return nc


def _get_nc():
    if "nc" not in _NC_CACHE:
        _NC_CACHE["nc"] = _build_bass()
    return _NC_CACHE["nc"]


def _prep_in_maps(f, g, W1t, b1t, W1p, b1p, W2, b2):
    f2 = np.asarray(f, np.float32).reshape(BATCH, TRANS_H)
    g2 = np.asarray(g, np.float32).reshape(BATCH, PRED_H)

    w1 = np.zeros((K_PAD, JOINT_H), np.float32)
    w1[:TRANS_H] = np.asarray(W1t, np.float32).T
    w1[TRANS_H:K_TOTAL] = np.asarray(W1p, np.float32).T
    b1 = (np.asarray(b1t, np.float32) + np.asarray(b1p, np.float32)).reshape(
        JOINT_H, 1
    )
    w2T = np.ascontiguousarray(np.asarray(W2, np.float32).T)
    b2c = np.asarray(b2, np.float32).reshape(NUM_LABELS, 1)

    in_maps = []
    for core in range(N_CORES):
        sl = slice(core * N_PER_CORE, (core + 1) * N_PER_CORE)
        xT = np.zeros((K_PAD, N_PER_CORE), np.float32)
        xT[:TRANS_H] = f2[sl].T
        xT[TRANS_H:K_TOTAL] = g2[sl].T
        in_maps.append(
            {"xT": xT, "w1": w1, "b1": b1, "w2T": w2T, "b2": b2c}
        )
    return in_maps


def _gather(results):
    y = np.empty((1, BATCH, NUM_LABELS), np.float32)
    for core, r in enumerate(results):
        y[0, core * N_PER_CORE:(core + 1) * N_PER_CORE] = r["yT"].T
    return y


def _run(inputs, trace=False):
    in_maps = _prep_in_maps(
        inputs["f"], inputs["g"], inputs["W1t"], inputs["b1t"],
        inputs["W1p"], inputs["b1p"], inputs["W2"], inputs["b2"],
    )
    res = run_bass_kernel_spmd(
        _get_nc(), in_maps, core_ids=list(range(N_CORES)), trace=trace
    )
    return _gather(res.results), res


def kernel(**inputs) -> np.ndarray:
    out, _ = _run(inputs, trace=False)
    return out



# revision 3
# speedup vs baseline: 1.2175x; 1.2175x over previous
"""RNN-T joint network (dense MLP) Trainium2 Bass kernel.

Math (per batch row n):
    h = relu(x @ W1.T + b1)     x = concat(f, g)   # [N, 512]
    y = h @ W2.T + b2                              # [N, 29]

Strategy: data-parallel over batch N=32768 across 8 NeuronCores (4096
rows/core); weights replicated.

Layer 1 runs on the PE in fp8e4 (e4m3) DoubleRow mode (0.5 cycles/row,
2 k-rows per slot-pair) with full 3-term error compensation so accuracy
stays at the bf16 level (~1.2e-3 max rel err):

    x ~= x8 + xlo          (both e4m3, exact split of the f32 value)
    W ~= (Whi + Wlo)/S     (e4m3 at device scale S=4096; Wlo = residual)
    x @ W = (x8 @ (Whi + Wlo) + xlo @ Whi) / S     [+ O(eps^2) dropped]

Every DoubleRow matmul carries two k-tiles (one per slot), so per j-tile
and 512-row chunk layer 1 is 16 DR instructions covering K=1344
(10 full 128-tiles + one 64-row tile packed by stacking [Whi;Wlo]
vertically in one slot) x 3 terms, at 256 PE cycles each — 4096 cycles
vs 5632 for the fp32r baseline.  The c-term (xlo @ Whi) reuses the
a-term weight tiles in SBUF.  All terms share PSUM scale S; the
ScalarE activation applies scale=1/S with the bias + ReLU.

Layer 2 stays fp32r (29-wide output, 4 matmuls/chunk).  Host prep packs
x8/xlo into one fp8 DRAM tensor per core (7.5MB vs 23MB f32 — DMA drops
well under the PE roofline).
"""

import numpy as np
import ml_dtypes

import concourse.bacc as bacc
import concourse.bass as bass  # noqa: F401
import concourse.mybir as mybir
from concourse import tile
from concourse.bass_utils import run_bass_kernel_spmd

TRANS_H, PRED_H, JOINT_H, NUM_LABELS = 1024, 320, 512, 29
BATCH = 32768
N_CORES = 8
N_PER_CORE = BATCH // N_CORES          # 4096
K_TOTAL = TRANS_H + PRED_H             # 1344 = 10*128 + 64
K_FULL = 10                            # full 128-row k-tiles
K_REM = K_TOTAL - K_FULL * 128         # 64
J_TILES = JOINT_H // 128               # 4
N_CHUNK = 512                          # PSUM-bank / fp32r moving limit
N_CHUNKS = N_PER_CORE // N_CHUNK       # 8
X_SLOTS = 22                           # 10 x8 + x8dup + xlohalf + 10 xlo
N_DR = 11                              # weight DR slots per j (a:5, b:5, t10:1)
W_SCALE = 4096.0

F32 = mybir.dt.float32
F32R = mybir.dt.float32r
F8 = mybir.dt.float8e4
DR = mybir.MatmulPerfMode.DoubleRow
E4NP = ml_dtypes.float8_e4m3

_NC_CACHE = {}


def _build_bass():
    """Single-core Bass program (same NEFF runs SPMD on 8 cores)."""
    nc = bacc.Bacc(None)

    xq = nc.dram_tensor("xq", [X_SLOTS * 128, N_PER_CORE], F8, kind="ExternalInput")
    w1 = nc.dram_tensor("w1", [128, J_TILES, N_DR, 2, 128], F8, kind="ExternalInput")
    b1 = nc.dram_tensor("b1", [JOINT_H, 1], F32, kind="ExternalInput")
    w2T = nc.dram_tensor("w2T", [JOINT_H, NUM_LABELS], F32R, kind="ExternalInput")
    b2 = nc.dram_tensor("b2", [NUM_LABELS, 1], F32, kind="ExternalInput")
    yT = nc.dram_tensor("yT", [NUM_LABELS, N_PER_CORE], F32, kind="ExternalOutput")

    xq3 = xq.rearrange("(s p) n -> p s n", p=128)     # [128, 22, N]

    with tile.TileContext(nc) as tc:
        with (
            tc.tile_pool(name="consts", bufs=1) as consts,
            tc.tile_pool(name="xpool", bufs=3) as xpool,
            tc.tile_pool(name="lopool", bufs=3) as lopool,
            tc.tile_pool(name="hpool", bufs=2) as hpool,
            tc.tile_pool(name="opool", bufs=2) as opool,
            tc.tile_pool(name="psum_h", bufs=6, space="PSUM") as psum_h,
            tc.tile_pool(name="psum_y", bufs=2, space="PSUM") as psum_y,
        ):
            # ---- replicated constants (ACT ring; x rides the SP ring) ----
            w2_sb = consts.tile([128, J_TILES, NUM_LABELS], F32R, name="w2_sb", tag="w2")
            nc.scalar.dma_start(
                out=w2_sb, in_=w2T.rearrange("(j p) l -> p j l", p=128)
            )
            b1_sb = consts.tile([128, J_TILES], F32, name="b1_sb", tag="b1")
            nc.scalar.dma_start(
                out=b1_sb, in_=b1.rearrange("(j p) o -> p (j o)", p=128)
            )
            b2_sb = consts.tile([NUM_LABELS, 1], F32, name="b2_sb", tag="b2")
            nc.scalar.dma_start(out=b2_sb, in_=b2[:, :])
            w1_sb = consts.tile([128, J_TILES, N_DR, 2, 128], F8, name="w1_sb", tag="w1")
            for j in range(J_TILES):
                nc.scalar.dma_start(out=w1_sb[:, j], in_=w1[:, j])

            # ---- main loop over batch chunks of 512 rows ----
            for c in range(N_CHUNKS):
                n0 = c * N_CHUNK
                x8_sb = xpool.tile([128, 12, N_CHUNK], F8, name="x8_sb", tag="x8")
                nc.sync.dma_start(out=x8_sb, in_=xq3[:, 0:12, n0:n0 + N_CHUNK])
                xlo_sb = lopool.tile([128, 10, N_CHUNK], F8, name="xlo_sb", tag="xlo")
                nc.sync.dma_start(out=xlo_sb, in_=xq3[:, 12:22, n0:n0 + N_CHUNK])

                h_tiles = []
                for j in range(J_TILES):
                    ph = psum_h.tile([128, N_CHUNK], F32, name=f"ph_{j}", tag="ph")
                    # a-terms: x8 @ Whi, 5 DR pairs over k-tiles 0..9
                    for q in range(5):
                        nc.tensor.matmul(
                            ph, lhsT=w1_sb[:, j, q], rhs=x8_sb[:, 2 * q:2 * q + 2, :],
                            start=(q == 0), stop=False, perf_mode=DR,
                        )
                    # b-terms: x8 @ Wlo
                    for q in range(5):
                        nc.tensor.matmul(
                            ph, lhsT=w1_sb[:, j, 5 + q], rhs=x8_sb[:, 2 * q:2 * q + 2, :],
                            start=False, stop=False, perf_mode=DR,
                        )
                    # k-tile 10 (64 rows): slot0 [Whi10;Wlo10]@[x8;x8],
                    # slot1 [Whi10;0]@[xlo;0]
                    nc.tensor.matmul(
                        ph, lhsT=w1_sb[:, j, 10], rhs=x8_sb[:, 10:12, :],
                        start=False, stop=False, perf_mode=DR,
                    )
                    # c-terms: xlo @ Whi (reuses a-term weight tiles)
                    for q in range(5):
                        nc.tensor.matmul(
                            ph, lhsT=w1_sb[:, j, q], rhs=xlo_sb[:, 2 * q:2 * q + 2, :],
                            start=False, stop=(q == 4), perf_mode=DR,
                        )
                    h_sb = hpool.tile([128, N_CHUNK], F32R, name=f"h_{j}", tag=f"h_{j}")
                    nc.scalar.activation(
                        h_sb, ph, mybir.ActivationFunctionType.Relu,
                        bias=b1_sb[:, j:j + 1], scale=1.0 / W_SCALE,
                    )
                    h_tiles.append(h_sb)

                py = psum_y.tile([NUM_LABELS, N_CHUNK], F32, name="py", tag="py")
                for j in range(J_TILES):
                    nc.tensor.matmul(
                        py, lhsT=w2_sb[:, j, :], rhs=h_tiles[j],
                        start=(j == 0), stop=(j == J_TILES - 1),
                    )
                y_sb = opool.tile([NUM_LABELS, N_CHUNK], F32, name="y_sb", tag="y")
                nc.scalar.activation(
                    y_sb, py, mybir.ActivationFunctionType.Identity, bias=b2_sb
                )
                nc.scalar.dma_start(out=yT[:, n0:n0 + N_CHUNK], in_=y_sb)

    nc.finalize()
    return nc


def _get_nc():
    if "nc" not in _NC_CACHE:
        _NC_CACHE["nc"] = _build_bass()
    return _NC_CACHE["nc"]


def _q8(a):
    return np.asarray(a, dtype=E4NP)


def _prep_in_maps(f, g, W1t, b1t, W1p, b1p, W2, b2):
    f2 = np.asarray(f, np.float32).reshape(BATCH, TRANS_H)
    g2 = np.asarray(g, np.float32).reshape(BATCH, PRED_H)
    x = np.concatenate([f2, g2], axis=1)            # [BATCH, 1344]

    x8 = _q8(x)                                     # e4m3, device scale 1
    xlo = _q8(x - x8.astype(np.float32))            # e4m3 residual, scale 1

    W1 = np.concatenate(
        [np.asarray(W1t, np.float32), np.asarray(W1p, np.float32)], axis=1
    ).T                                             # [1344, 512]
    Whi = _q8(W1 * W_SCALE)                         # device scale 4096
    Wlo = _q8(W1 * W_SCALE - Whi.astype(np.float32))

    # weight DR-pair tensor [p, j, dr, slot, col]
    w1dr = np.zeros((128, J_TILES, N_DR, 2, 128), dtype=E4NP)
    Whi_p = np.zeros((11 * 128, JOINT_H), dtype=E4NP)
    Wlo_p = np.zeros((11 * 128, JOINT_H), dtype=E4NP)
    Whi_p[:K_TOTAL] = Whi
    Wlo_p[:K_TOTAL] = Wlo
    for j in range(J_TILES):
        cols = slice(j * 128, (j + 1) * 128)
        for q in range(5):
            w1dr[:, j, q, 0] = Whi_p[(2 * q) * 128:(2 * q + 1) * 128, cols]
            w1dr[:, j, q, 1] = Whi_p[(2 * q + 1) * 128:(2 * q + 2) * 128, cols]
            w1dr[:, j, 5 + q, 0] = Wlo_p[(2 * q) * 128:(2 * q + 1) * 128, cols]
            w1dr[:, j, 5 + q, 1] = Wlo_p[(2 * q + 1) * 128:(2 * q + 2) * 128, cols]
        # k-tile 10: slot0 = [Whi10; Wlo10] stacked, slot1 = [Whi10; 0]
        w1dr[:K_REM, j, 10, 0] = Whi[K_FULL * 128:, cols]
        w1dr[K_REM:2 * K_REM, j, 10, 0] = Wlo[K_FULL * 128:, cols]
        w1dr[:K_REM, j, 10, 1] = Whi[K_FULL * 128:, cols]

    b1c = (np.asarray(b1t, np.float32) + np.asarray(b1p, np.float32)).reshape(
        JOINT_H, 1
    )
    w2c = np.ascontiguousarray(np.asarray(W2, np.float32).T)
    b2c = np.asarray(b2, np.float32).reshape(NUM_LABELS, 1)

    in_maps = []
    for core in range(N_CORES):
        sl = slice(core * N_PER_CORE, (core + 1) * N_PER_CORE)
        x8c = x8[sl]                                # [4096, 1344]
        xloc = xlo[sl]
        xqc = np.zeros((X_SLOTS, 128, N_PER_CORE), dtype=E4NP)
        for t in range(K_FULL):
            xqc[t] = x8c[:, t * 128:(t + 1) * 128].T
        # slot 10: x8 tile-10 duplicated vertically; slot 11: xlo tile-10 + 0s
        xqc[10, :K_REM] = x8c[:, K_FULL * 128:].T
        xqc[10, K_REM:] = x8c[:, K_FULL * 128:].T
        xqc[11, :K_REM] = xloc[:, K_FULL * 128:].T
        for t in range(K_FULL):
            xqc[12 + t] = xloc[:, t * 128:(t + 1) * 128].T
        in_maps.append({
            "xq": xqc.reshape(X_SLOTS * 128, N_PER_CORE),
            "w1": w1dr, "b1": b1c, "w2T": w2c, "b2": b2c,
        })
    return in_maps


def _gather(results):
    y = np.empty((1, BATCH, NUM_LABELS), np.float32)
    for core, r in enumerate(results):
        y[0, core * N_PER_CORE:(core + 1) * N_PER_CORE] = r["yT"].T
    return y


def _run(inputs, trace=False):
    in_maps = _prep_in_maps(
        inputs["f"], inputs["g"], inputs["W1t"], inputs["b1t"],
        inputs["W1p"], inputs["b1p"], inputs["W2"], inputs["b2"],
    )
    res = run_bass_kernel_spmd(
        _get_nc(), in_maps, core_ids=list(range(N_CORES)), trace=trace
    )
    return _gather(res.results), res


def kernel(**inputs) -> np.ndarray:
    out, _ = _run(inputs, trace=False)
    return out


# revision 5
# speedup vs baseline: 1.3267x; 1.0897x over previous
"""RNN-T joint network (dense MLP) Trainium2 Bass kernel.

Math (per batch row n):
    h = relu(x @ W1.T + b1)     x = concat(f, g)   # [N, 512]
    y = h @ W2.T + b2                              # [N, 29]

Strategy: data-parallel over batch N=32768 across 8 NeuronCores (4096
rows/core); weights replicated.

Layer 1 runs on the PE in fp8e4 (e4m3) DoubleRow mode (0.5 cycles/row,
2 k-rows per slot-pair) with full 3-term error compensation so accuracy
stays at the bf16 level (~1.2e-3 max rel err):

    x ~= x8 + xlo          (both e4m3, exact split of the f32 value)
    W ~= (Whi + Wlo)/S     (e4m3 at device scale S=4096; Wlo = residual)
    x @ W = (x8 @ (Whi + Wlo) + xlo @ Whi) / S     [+ O(eps^2) dropped]

Every DoubleRow matmul carries two k-tiles (one per slot), so per j-tile
and 512-row chunk layer 1 is 16 DR instructions covering K=1344
(10 full 128-tiles + one 64-row tile packed by stacking [Whi;Wlo]
vertically in one slot) x 3 terms, at 256 PE cycles each — 4096 cycles
vs 5632 for the fp32r baseline.  The c-term (xlo @ Whi) reuses the
a-term weight tiles in SBUF.  All terms share PSUM scale S; the
ScalarE activation applies scale=1/S with the bias + ReLU.

Layer 2 stays fp32r (29-wide output, 4 matmuls/chunk).  Host prep packs
x8/xlo into one fp8 DRAM tensor per core (7.5MB vs 23MB f32 — DMA drops
well under the PE roofline).
"""

import numpy as np
import ml_dtypes

import concourse.bacc as bacc
import concourse.bass as bass  # noqa: F401
import concourse.mybir as mybir
from concourse import tile
from concourse.bass_utils import run_bass_kernel_spmd

TRANS_H, PRED_H, JOINT_H, NUM_LABELS = 1024, 320, 512, 29
BATCH = 32768
N_CORES = 8
N_PER_CORE = BATCH // N_CORES          # 4096
K_TOTAL = TRANS_H + PRED_H             # 1344 = 10*128 + 64
K_FULL = 10                            # full 128-row k-tiles
K_REM = K_TOTAL - K_FULL * 128         # 64
J_TILES = JOINT_H // 128               # 4
N_CHUNK = 512                          # PSUM-bank / fp32r moving limit
N_CHUNKS = N_PER_CORE // N_CHUNK       # 8
X_SLOTS = 22                           # 10 x8 + x8dup + xlohalf + 10 xlo
N_DR = 11                              # weight DR slots per j (a:5, b:5, t10:1)
W_SCALE = 4096.0

F32 = mybir.dt.float32
F32R = mybir.dt.float32r
F8 = mybir.dt.float8e4
DR = mybir.MatmulPerfMode.DoubleRow
E4NP = ml_dtypes.float8_e4m3

_NC_CACHE = {}


def _build_bass():
    """Single-core Bass program (same NEFF runs SPMD on 8 cores)."""
    nc = bacc.Bacc(None)

    xq = nc.dram_tensor("xq", [X_SLOTS * 128, N_PER_CORE], F8, kind="ExternalInput")
    w1 = nc.dram_tensor("w1", [128, J_TILES, N_DR, 2, 128], F8, kind="ExternalInput")
    b1 = nc.dram_tensor("b1", [JOINT_H, 1], F32, kind="ExternalInput")
    w2T = nc.dram_tensor("w2T", [JOINT_H, NUM_LABELS], F32R, kind="ExternalInput")
    b2 = nc.dram_tensor("b2", [NUM_LABELS, 1], F32, kind="ExternalInput")
    yT = nc.dram_tensor("yT", [NUM_LABELS, N_PER_CORE], F32, kind="ExternalOutput")

    xq3 = xq.rearrange("(s p) n -> p s n", p=128)     # [128, 22, N]

    with tile.TileContext(nc) as tc:
        with (
            tc.tile_pool(name="consts", bufs=1) as consts,
            tc.tile_pool(name="xpool", bufs=3) as xpool,
            tc.tile_pool(name="lopool", bufs=3) as lopool,
            tc.tile_pool(name="hpool", bufs=2) as hpool,
            tc.tile_pool(name="opool", bufs=2) as opool,
            tc.tile_pool(name="psum_h", bufs=6, space="PSUM") as psum_h,
            tc.tile_pool(name="psum_y", bufs=2, space="PSUM") as psum_y,
        ):
            # ---- replicated constants ----
            # w1 j0 rides FIRST on the SP ring so the PE can start ~3us in;
            # everything else follows on the ACT ring.
            w1_sb = consts.tile([128, J_TILES, N_DR, 2, 128], F8, name="w1_sb", tag="w1")
            nc.sync.dma_start(out=w1_sb[:, 0], in_=w1[:, 0])
            w2_sb = consts.tile([128, J_TILES, NUM_LABELS], F32R, name="w2_sb", tag="w2")
            b1_sb = consts.tile([128, J_TILES], F32, name="b1_sb", tag="b1")
            nc.scalar.dma_start(
                out=b1_sb, in_=b1.rearrange("(j p) o -> p (j o)", p=128)
            )
            b2_sb = consts.tile([NUM_LABELS, 1], F32, name="b2_sb", tag="b2")
            for j in range(1, J_TILES):
                nc.scalar.dma_start(out=w1_sb[:, j], in_=w1[:, j])
            nc.scalar.dma_start(
                out=w2_sb, in_=w2T.rearrange("(j p) l -> p j l", p=128)
            )
            nc.scalar.dma_start(out=b2_sb, in_=b2[:, :])

            # ---- main loop over batch chunks of 512 rows ----
            # Layer 2 of chunk c-1 is emitted after chunk c's first DR block
            # so the PE never idles waiting on the j3 activation (tiny gaps
            # reset the p-state ramp and cost ~3us of half-clock matmuls).
            pending = None  # (h_tiles, n0) of previous chunk
            for c in range(N_CHUNKS):
                n0 = c * N_CHUNK
                x8_sb = xpool.tile([128, 12, N_CHUNK], F8, name="x8_sb", tag="x8")
                nc.sync.dma_start(out=x8_sb, in_=xq3[:, 0:12, n0:n0 + N_CHUNK])
                xlo_sb = lopool.tile([128, 10, N_CHUNK], F8, name="xlo_sb", tag="xlo")
                nc.sync.dma_start(out=xlo_sb, in_=xq3[:, 12:22, n0:n0 + N_CHUNK])

                h_tiles = []
                for j in range(J_TILES):
                    ph = psum_h.tile([128, N_CHUNK], F32, name=f"ph_{j}", tag="ph")
                    # a-terms: x8 @ Whi, 5 DR pairs over k-tiles 0..9
                    for q in range(5):
                        nc.tensor.matmul(
                            ph, lhsT=w1_sb[:, j, q], rhs=x8_sb[:, 2 * q:2 * q + 2, :],
                            start=(q == 0), stop=False, perf_mode=DR,
                        )
                    # b-terms: x8 @ Wlo
                    for q in range(5):
                        nc.tensor.matmul(
                            ph, lhsT=w1_sb[:, j, 5 + q], rhs=x8_sb[:, 2 * q:2 * q + 2, :],
                            start=False, stop=False, perf_mode=DR,
                        )
                    # k-tile 10 (64 rows): slot0 [Whi10;Wlo10]@[x8;x8],
                    # slot1 [Whi10;0]@[xlo;0]
                    nc.tensor.matmul(
                        ph, lhsT=w1_sb[:, j, 10], rhs=x8_sb[:, 10:12, :],
                        start=False, stop=False, perf_mode=DR,
                    )
                    # c-terms: xlo @ Whi (reuses a-term weight tiles)
                    for q in range(5):
                        nc.tensor.matmul(
                            ph, lhsT=w1_sb[:, j, q], rhs=xlo_sb[:, 2 * q:2 * q + 2, :],
                            start=False, stop=(q == 4), perf_mode=DR,
                        )
                    h_sb = hpool.tile([128, N_CHUNK], F32R, name=f"h_{j}", tag=f"h_{j}")
                    nc.scalar.activation(
                        h_sb, ph, mybir.ActivationFunctionType.Relu,
                        bias=b1_sb[:, j:j + 1], scale=1.0 / W_SCALE,
                    )
                    h_tiles.append(h_sb)
                    if j == 0 and pending is not None:
                        _emit_layer2(nc, psum_y, opool, w2_sb, b2_sb, yT, *pending)
                        pending = None

                pending = (h_tiles, n0)
            _emit_layer2(nc, psum_y, opool, w2_sb, b2_sb, yT, *pending)

    nc.finalize()
    return nc


def _emit_layer2(nc, psum_y, opool, w2_sb, b2_sb, yT, h_tiles, n0):
    py = psum_y.tile([NUM_LABELS, N_CHUNK], F32, name="py", tag="py")
    for j in range(J_TILES):
        nc.tensor.matmul(
            py, lhsT=w2_sb[:, j, :], rhs=h_tiles[j],
            start=(j == 0), stop=(j == J_TILES - 1),
        )
    y_sb = opool.tile([NUM_LABELS, N_CHUNK], F32, name="y_sb", tag="y")
    nc.scalar.activation(
        y_sb, py, mybir.ActivationFunctionType.Identity, bias=b2_sb
    )
    nc.scalar.dma_start(out=yT[:, n0:n0 + N_CHUNK], in_=y_sb)


def _get_nc():
    if "nc" not in _NC_CACHE:
        _NC_CACHE["nc"] = _build_bass()
    return _NC_CACHE["nc"]


def _q8(a):
    return np.asarray(a, dtype=E4NP)


def _prep_in_maps(f, g, W1t, b1t, W1p, b1p, W2, b2):
    f2 = np.asarray(f, np.float32).reshape(BATCH, TRANS_H)
    g2 = np.asarray(g, np.float32).reshape(BATCH, PRED_H)
    x = np.concatenate([f2, g2], axis=1)            # [BATCH, 1344]

    x8 = _q8(x)                                     # e4m3, device scale 1
    xlo = _q8(x - x8.astype(np.float32))            # e4m3 residual, scale 1

    W1 = np.concatenate(
        [np.asarray(W1t, np.float32), np.asarray(W1p, np.float32)], axis=1
    ).T                                             # [1344, 512]
    Whi = _q8(W1 * W_SCALE)                         # device scale 4096
    Wlo = _q8(W1 * W_SCALE - Whi.astype(np.float32))

    # weight DR-pair tensor [p, j, dr, slot, col]
    w1dr = np.zeros((128, J_TILES, N_DR, 2, 128), dtype=E4NP)
    Whi_p = np.zeros((11 * 128, JOINT_H), dtype=E4NP)
    Wlo_p = np.zeros((11 * 128, JOINT_H), dtype=E4NP)
    Whi_p[:K_TOTAL] = Whi
    Wlo_p[:K_TOTAL] = Wlo
    for j in range(J_TILES):
        cols = slice(j * 128, (j + 1) * 128)
        for q in range(5):
            w1dr[:, j, q, 0] = Whi_p[(2 * q) * 128:(2 * q + 1) * 128, cols]
            w1dr[:, j, q, 1] = Whi_p[(2 * q + 1) * 128:(2 * q + 2) * 128, cols]
            w1dr[:, j, 5 + q, 0] = Wlo_p[(2 * q) * 128:(2 * q + 1) * 128, cols]
            w1dr[:, j, 5 + q, 1] = Wlo_p[(2 * q + 1) * 128:(2 * q + 2) * 128, cols]
        # k-tile 10: slot0 = [Whi10; Wlo10] stacked, slot1 = [Whi10; 0]
        w1dr[:K_REM, j, 10, 0] = Whi[K_FULL * 128:, cols]
        w1dr[K_REM:2 * K_REM, j, 10, 0] = Wlo[K_FULL * 128:, cols]
        w1dr[:K_REM, j, 10, 1] = Whi[K_FULL * 128:, cols]

    b1c = (np.asarray(b1t, np.float32) + np.asarray(b1p, np.float32)).reshape(
        JOINT_H, 1
    )
    w2c = np.ascontiguousarray(np.asarray(W2, np.float32).T)
    b2c = np.asarray(b2, np.float32).reshape(NUM_LABELS, 1)

    in_maps = []
    for core in range(N_CORES):
        sl = slice(core * N_PER_CORE, (core + 1) * N_PER_CORE)
        x8c = x8[sl]                                # [4096, 1344]
        xloc = xlo[sl]
        xqc = np.zeros((X_SLOTS, 128, N_PER_CORE), dtype=E4NP)
        for t in range(K_FULL):
            xqc[t] = x8c[:, t * 128:(t + 1) * 128].T
        # slot 10: x8 tile-10 duplicated vertically; slot 11: xlo tile-10 + 0s
        xqc[10, :K_REM] = x8c[:, K_FULL * 128:].T
        xqc[10, K_REM:] = x8c[:, K_FULL * 128:].T
        xqc[11, :K_REM] = xloc[:, K_FULL * 128:].T
        for t in range(K_FULL):
            xqc[12 + t] = xloc[:, t * 128:(t + 1) * 128].T
        in_maps.append({
            "xq": xqc.reshape(X_SLOTS * 128, N_PER_CORE),
            "w1": w1dr, "b1": b1c, "w2T": w2c, "b2": b2c,
        })
    return in_maps


def _gather(results):
    y = np.empty((1, BATCH, NUM_LABELS), np.float32)
    for core, r in enumerate(results):
        y[0, core * N_PER_CORE:(core + 1) * N_PER_CORE] = r["yT"].T
    return y


def _run(inputs, trace=False):
    in_maps = _prep_in_maps(
        inputs["f"], inputs["g"], inputs["W1t"], inputs["b1t"],
        inputs["W1p"], inputs["b1p"], inputs["W2"], inputs["b2"],
    )
    res = run_bass_kernel_spmd(
        _get_nc(), in_maps, core_ids=list(range(N_CORES)), trace=trace
    )
    return _gather(res.results), res


def kernel(**inputs) -> np.ndarray:
    out, _ = _run(inputs, trace=False)
    return out
